# revision 4
# baseline (speedup 1.0000x reference)
"""Difference-attention Trainium2 kernel v2: low-rank separable score.

K(x,y) = exp(-|x-y|) ~= Phi(x)^T W Psi(y) with one-hw-op feature families:
  Psi (k-side, per-feature cost on the big [128,1408] tensor):
      [k, k^2, e^k, e^-k, tanh(s_b(k-c_b)) x6] on ACT, [|k-c_b|] x4 on DVE.
  Phi (q-side, cheap [128,64] evals): [1, q, q^2, e^q, e^-q, tanh x14,
      |q-c| x8], mixed into per-k-feature coefficient tiles by constant
      block-diagonal PE matmuls (W fitted at import; pure numpy).
Coord term: same machinery per dim d with Psi_co = [kc, |kc-c| x12],
  Phi_co = [1, qc, |qc-t| x10], wsum_d folded in at runtime.
Score accumulates in psum[128=(half,query), 1408]; tail: exp -> PE
transposes -> attn@[v|1] -> divide -> Wproj.
"""

import sys

if "/opt/trn_rl_repo" not in sys.path:
    sys.path.insert(0, "/opt/trn_rl_repo")

import numpy as np
import ml_dtypes

B = 1
NQ = 512
NKV = 2562
C = 64
ICO = 64
NCORES = 8
NQL = NQ // NCORES
NJP = 2816
NH = NJP // 2
NT = NJP // 128
NCH = NH // 128
SCALE = 1.0 / C

# ---- approximation parameters (offline joint optimization) ----
PSI_TANH = [(1.6021459636894778, -0.13161878574255414),
            (1.0973968672906501, -2.357533148689835),
            (1.0845020969105885, 2.1082036571274),
            (2.516242434966713, 0.029135138791623184),
            (1.1364504325228512, -1.112283710208587),
            (1.1891793083360349, 1.1313633739887903)]
PSI_ABS = [-2.939287325119229, -0.8391264764544882,
           0.906817365296043, 2.9751008489413425]
PHI_TANH_C = list(np.linspace(-5.2, 4.6, 14))
PHI_TANH_S = 1.1
PHI_ABS_C = list(np.linspace(-4.5, 4.0, 8))
CO_ABS = list(np.linspace(-3.8, 3.8, 12))
CO_QNODES = list(np.linspace(-3.2, 3.2, 10))

SK = 4 + len(PSI_TANH) + len(PSI_ABS)          # 14 k-features
# q-side slot layout (28 slots = 14 parity packs of 2; slot 3 is a pad):
#   0:'1' 1:'q' | 2:'q2' 3:pad | 4:'e+' 5:'e-' | 6..19 tanh | 20..27 abs
SQSLOT = 28
NQP = SQSLOT // 2                               # 14 packs
SKC = len(CO_ABS)                               # 12 coord k-features (abs)
NCOR = 3 * SKC                                  # 36 rows per half
# coord q-side rows: [0:30] = |qc-t| x10, [32:35] = ones; K = 35
NCOQ = 35
NKP = SK // 2                                   # 7 mixing out-packs


def _psi_eval(x):
    cols = [x, x * x,
            np.exp(np.clip(x, None, 6.0)), np.exp(-np.clip(x, -6.0, None))]
    for s, c in PSI_TANH:
        cols.append(np.tanh(s * (x - c)))
    for c in PSI_ABS:
        cols.append(np.abs(x - c))
    return np.stack(cols, -1)


def _phi_eval_slots(x):
    """q-side features in SLOT order (28 cols, slot 3 = zero pad)."""
    cols = [np.ones_like(x), x, x * x, np.zeros_like(x),
            np.exp(np.clip(x, None, 6.2)), np.exp(-np.clip(x, -6.6, None))]
    for c in PHI_TANH_C:
        cols.append(np.tanh(PHI_TANH_S * (x - c)))
    for c in PHI_ABS_C:
        cols.append(np.abs(x - c))
    return np.stack(cols, -1)


def _fit_weights():
    qs = np.linspace(-6.6, 5.6, 199)
    ks = np.linspace(-6.0, 6.0, 221)
    wq = np.exp(-qs ** 2 / 8) + 0.25
    wk = np.exp(-ks ** 2 / 2) + 1e-3
    swq, swk = np.sqrt(wq), np.sqrt(wk)
    KER = np.exp(-np.abs(qs[:, None] - ks[None, :]))
    KW = swq[:, None] * KER * swk[None, :]

    def solve(P, S, lam=3e-6):
        pn = np.sqrt((P ** 2).mean(0))
        pn[pn == 0] = 1.0
        sn = np.sqrt((S ** 2).mean(0))
        Pn, Sn = P / pn, S / sn
        A = np.linalg.solve(Pn.T @ Pn + lam * len(qs) * np.eye(Pn.shape[1]),
                            Pn.T @ KW)
        Wn = np.linalg.solve(Sn.T @ Sn + lam * len(ks) * np.eye(Sn.shape[1]),
                             Sn.T @ A.T).T
        return Wn / pn[:, None] / sn[None, :]

    # channel fit: use slot-phi WITHOUT the pad column, reinsert after
    P_all = _phi_eval_slots(qs)
    keep = [i for i in range(SQSLOT) if i != 3]
    P = P_all[:, keep] * swq[:, None]
    S_full = np.concatenate([np.ones((len(ks), 1)), _psi_eval(ks)], axis=1)
    Wfit = solve(P, S_full * swk[:, None])[:, 1:]     # drop free const
    W = np.zeros((SQSLOT, SK), np.float32)
    W[keep, :] = Wfit

    Pc = np.stack([np.ones_like(qs)] + [np.abs(qs - t) for t in CO_QNODES],
                  -1)
    Sc = np.stack([np.ones_like(ks)] + [np.abs(ks - c) for c in CO_ABS], -1)
    CWfull = solve(Pc * swq[:, None], Sc * swk[:, None])
    CW = CWfull[:, 1:]                                # drop free const
    return W.astype(np.float32), CW.astype(np.float32)


_W_MIX, _CW_MIX = _fit_weights()
_COMPILED = None


def _build_program():
    import concourse.bass as bass
    import concourse.tile as tile
    from concourse import bacc, mybir

    F32 = mybir.dt.float32
    BF16 = mybir.dt.bfloat16
    U16 = mybir.dt.uint16
    U32 = mybir.dt.uint32
    ALU = mybir.AluOpType
    ACTF = mybir.ActivationFunctionType
    AX = mybir.AxisListType
    PSUM = bass.MemorySpace.PSUM
    BF = ml_dtypes.bfloat16

    nc = bacc.Bacc("TRN2", target_bir_lowering=False, debug=False,
                   num_devices=NCORES)

    q_d = nc.dram_tensor("q", [NQL, C], F32, kind="ExternalInput")
    qc_d = nc.dram_tensor("q_coord", [NQL, 3], F32, kind="ExternalInput")
    kv_d = nc.dram_tensor("kv", [NKV, ICO], F32, kind="ExternalInput")
    kvc_d = nc.dram_tensor("kv_coord", [NKV, 3], F32, kind="ExternalInput")
    wq_d = nc.dram_tensor("Wq", [C, C], F32, kind="ExternalInput")
    wkv_d = nc.dram_tensor("Wkv", [ICO, 2 * C], F32, kind="ExternalInput")
    wd_d = nc.dram_tensor("Wdelta", [3, C], F32, kind="ExternalInput")
    wp_d = nc.dram_tensor("Wproj", [C, C], F32, kind="ExternalInput")
    bp_d = nc.dram_tensor("bproj", [C, 1], F32, kind="ExternalInput")
    out_d = nc.dram_tensor("out", [NQL, C], F32, kind="ExternalOutput")

    ident_d = nc.inline_tensor(np.eye(128, dtype=np.float32), name="ident")

    # mixing weights laid [128, NQP*NKP*128]: block (p,r) at col (p*NKP+r)*128
    blocks = np.zeros((128, NQP * NKP * 128), dtype=np.float32)
    for p in range(NQP):
        for mp in range(2):
            m = 2 * p + mp
            for r in range(NKP):
                for npar in range(2):
                    n = 2 * r + npar
                    if n >= SK:
                        continue
                    v = _W_MIX[m, n]
                    base = (r * NQP + p) * 128
                    for c in range(C):
                        blocks[64 * mp + c, base + 64 * npar + c] = v
    bigw_d = nc.inline_tensor(blocks, name="bigw")

    # coord mixing block [NCOQ=35, NCOR]: q-rows = abs 3m+d (m<10), ones 32+d
    cwblk = np.zeros((NCOQ, NCOR), dtype=np.float32)
    for n in range(SKC):
        for d in range(3):
            for m in range(len(CO_QNODES)):
                cwblk[3 * m + d, 3 * n + d] = _CW_MIX[1 + m, n]
            cwblk[32 + d, 3 * n + d] = _CW_MIX[0, n]
    cw_d = nc.inline_tensor(cwblk, name="cwblk")

    def colvec(lo, hi):
        a = np.zeros((128, 1), np.float32)
        a[0:64, 0] = lo
        a[64:128, 0] = hi
        return a

    exp_scale_d = nc.inline_tensor(colvec(1.0, -1.0), name="exps")
    tanh_s_d, tanh_b_d = [], []
    for i in range(7):
        c0, c1 = PHI_TANH_C[2 * i], PHI_TANH_C[2 * i + 1]
        s = PHI_TANH_S
        tanh_s_d.append(nc.inline_tensor(colvec(s, s), name=f"tns{i}"))
        tanh_b_d.append(nc.inline_tensor(colvec(-s * c0, -s * c1),
                                         name=f"tnb{i}"))
    abs_c_d = [nc.inline_tensor(colvec(PHI_ABS_C[2 * i], PHI_ABS_C[2 * i + 1]),
                                name=f"abc{i}") for i in range(4)]

    cok = np.zeros((128, 1), np.float32)
    for n in range(len(CO_ABS)):
        for d in range(3):
            cok[3 * n + d, 0] = CO_ABS[n]
            cok[64 + 3 * n + d, 0] = CO_ABS[n]
    cok_d = nc.inline_tensor(cok, name="cok")
    coq = np.zeros((128, 1), np.float32)
    for m in range(len(CO_QNODES)):
        for d in range(3):
            coq[3 * m + d, 0] = CO_QNODES[m]
    coq_d = nc.inline_tensor(coq, name="coq")
    # k-side tanh biases as [128,1] AP columns (float bias needs const AP)
    kpb = np.zeros((128, len(PSI_TANH)), np.float32)
    for i, (s, c) in enumerate(PSI_TANH):
        kpb[:, i] = -s * c
    kpb_d = nc.inline_tensor(kpb, name="kpb")

    s_q = nc.dram_tensor("s_q", [C, NQL], F32)
    s_kc = nc.dram_tensor("s_kc", [3, NJP], BF16)
    s_qc = nc.dram_tensor("s_qc", [3, NQL], F32)
    s_ws = nc.dram_tensor("s_ws", [3, 1], F32)

    with tile.TileContext(nc) as tc:
        with (
            tc.tile_pool(name="consts", bufs=1) as cp,
            tc.tile_pool(name="big", bufs=1) as bigp,
            tc.tile_pool(name="work", bufs=3) as wp,
        ):
            # ------------- kv DMA first -------------
            kv_sb = bigp.tile([128, NT * ICO], F32, tag="kv_sb")
            nfull = NT - 2
            kv3 = kv_d.ap()[0:nfull * 128, :].rearrange("(t p) c -> p t c",
                                                        p=128)
            qeng = (nc.sync, nc.scalar)
            for s4 in range(4):
                t0, t1 = 5 * s4, min(5 * (s4 + 1), nfull)
                qeng[s4 % 2].dma_start(kv_sb[:, t0 * ICO:t1 * ICO],
                                       kv3[:, t0:t1, :])
            nc.vector.memset(kv_sb[:, nfull * ICO:NT * ICO], 0.0)
            nc.sync.dma_start(kv_sb[0:NKV - nfull * 128,
                                    nfull * ICO:nfull * ICO + ICO],
                              kv_d.ap()[nfull * 128:NKV, :])

            # ------------- constants -------------
            ident = cp.tile([128, 128], F32, tag="ident")
            nc.scalar.dma_start(ident[:], ident_d.ap())
            identb = cp.tile([128, 128], BF16, tag="identb")
            nc.vector.tensor_copy(identb[:], ident[:])
            wq = cp.tile([C, C], F32, tag="wq")
            nc.scalar.dma_start(wq[:], wq_d.ap())
            wkv = cp.tile([ICO, 2 * C], F32, tag="wkv")
            nc.sync.dma_start(wkv[:], wkv_d.ap())
            wkv_hi = cp.tile([128, 2 * C], F32, tag="wkv_hi")
            nc.sync.dma_start(wkv_hi[64:128, :], wkv_d.ap())
            wkvkz = cp.tile([ICO, 128], F32, tag="wkvkz")
            nc.vector.memset(wkvkz[:, 0:C], 0.0)
            nc.sync.dma_start(wkvkz[:, C:2 * C], wkv_d.ap()[:, 0:C])
            wkvkz_hi = cp.tile([128, 128], F32, tag="wkvkz_hi")
            nc.vector.memset(wkvkz_hi[64:128, 0:C], 0.0)
            nc.sync.dma_start(wkvkz_hi[64:128, C:2 * C], wkv_d.ap()[:, 0:C])
            wproj = cp.tile([C, C], F32, tag="wproj")
            nc.scalar.dma_start(wproj[:], wp_d.ap())
            bproj = cp.tile([C, 1], F32, tag="bproj")
            nc.scalar.dma_start(bproj[:], bp_d.ap())
            wd = cp.tile([3, C], F32, tag="wd")
            nc.scalar.dma_start(wd[:], wd_d.ap())
            bigw = cp.tile([128, NQP * NKP * 128], F32, tag="bigw")
            bw_cols = NQP * NKP * 128
            rg = NQP * 128
            for r in range(NKP):
                nc.gpsimd.dma_start(bigw[:, r * rg:(r + 1) * rg],
                                    bigw_d.ap()[:, r * rg:(r + 1) * rg])
            cwb = cp.tile([NCOQ, NCOR], F32, tag="cwb")
            nc.scalar.dma_start(cwb[:], cw_d.ap())
            exps = cp.tile([128, 1], F32, tag="exps")
            nc.scalar.dma_start(exps[:], exp_scale_d.ap())
            tnsb = []
            for i in range(7):
                ts_ = cp.tile([128, 1], F32, tag=f"tns{i}", name=f"tns{i}")
                tb_ = cp.tile([128, 1], F32, tag=f"tnb{i}", name=f"tnb{i}")
                nc.scalar.dma_start(ts_[:], tanh_s_d[i].ap())
                nc.scalar.dma_start(tb_[:], tanh_b_d[i].ap())
                tnsb.append((ts_, tb_))
            absc = []
            for i in range(4):
                ac_ = cp.tile([128, 1], F32, tag=f"abc{i}", name=f"abc{i}")
                nc.scalar.dma_start(ac_[:], abs_c_d[i].ap())
                absc.append(ac_)
            cokt = cp.tile([128, 1], F32, tag="cok")
            nc.scalar.dma_start(cokt[:], cok_d.ap())
            coqt = cp.tile([128, 1], F32, tag="coq")
            nc.scalar.dma_start(coqt[:], coq_d.ap())
            kpbt = cp.tile([128, len(PSI_TANH)], F32, tag="kpb")
            nc.scalar.dma_start(kpbt[:], kpb_d.ap())

            wsum = cp.tile([3, 1], F32, tag="wsum")
            nc.vector.tensor_reduce(wsum[:], wd[:], axis=AX.X, op=ALU.add)
            nc.scalar.dma_start(s_ws.ap(), wsum[:])
            wrep = cp.tile([NCOR, 1], F32, tag="wrep")
            nc.scalar.dma_start(
                wrep[:], s_ws.ap().unsqueeze(0).broadcast_to([SKC, 3, 1]))

            # ------------- persistent tensors -------------
            kT2 = bigp.tile([128, NH], BF16, tag="kT2")
            vext = bigp.tile([128, NT * 65], BF16, tag="vext")
            qT2 = bigp.tile([128, NQL], F32, tag="qT2")
            gtiles = [bigp.tile([128, NH], BF16, tag=f"g{n}", name=f"g{n}")
                      for n in range(1, SK)]
            glist = [kT2] + gtiles
            gco = bigp.tile([128, NH], BF16, tag="gco")
            kcrep = bigp.tile([128, NH], BF16, tag="kcrep")
            e2sb = bigp.tile([128, NH], BF16, tag="e2sb")
            fq = [bigp.tile([128, 128], BF16, tag=f"fq{n}", name=f"fq{n}")
                  for n in range(SK)]
            aco = bigp.tile([128, 128], BF16, tag="aco")

            # ------------- phase A -------------
            with tc.tile_pool(name="psA", bufs=1, space=PSUM) as psA, \
                 tc.tile_pool(name="tmpA", bufs=1) as tmpA:
                # q path
                q_sb = tmpA.tile([NQL, C], F32, tag="q_sb")
                nc.sync.dma_start(q_sb[:], q_d.ap())
                p_qt = psA.tile([C, NQL], F32, tag="pq")
                nc.tensor.transpose(p_qt[:], q_sb[:], ident[0:NQL, 0:NQL])
                qt_sb = tmpA.tile([C, NQL], F32, tag="qt_sb")
                nc.scalar.copy(qt_sb[:], p_qt[:])
                p_q2 = psA.tile([C, NQL], F32, tag="pq")
                nc.tensor.matmul(p_q2[:], wq[:], qt_sb[:], start=True,
                                 stop=True)
                qta = tmpA.tile([C, NQL], F32, tag="qta")
                nc.scalar.copy(qta[:], p_q2[:])
                nc.sync.dma_start(s_q.ap(), qta[:])
                nc.scalar.dma_start(
                    qT2[:], s_q.ap().unsqueeze(0).broadcast_to([2, C, NQL]))

                # kv_coord path
                kvcT = tmpA.tile([3, NJP], F32, tag="kvcT")
                nc.sync.dma_start(kvcT[:, 0:NKV], kvc_d.ap().transpose([1, 0]))
                nc.vector.memset(kvcT[:, NKV:NJP], 0.0)
                kvcT_bf = tmpA.tile([3, NJP], BF16, tag="kvcTbf")
                nc.vector.tensor_copy(kvcT_bf[:], kvcT[:])
                nc.sync.dma_start(s_kc.ap(), kvcT_bf[:])
                nc.gpsimd.dma_start(
                    kcrep[0:NCOR, :],
                    s_kc.ap()[:, 0:NH].unsqueeze(0).broadcast_to(
                        [SKC, 3, NH]))
                nc.gpsimd.dma_start(
                    kcrep[64:64 + NCOR, :],
                    s_kc.ap()[:, NH:NJP].unsqueeze(0).broadcast_to(
                        [SKC, 3, NH]))

                # q_coord path
                qc_sb = tmpA.tile([NQL, 3], F32, tag="qc_sb")
                nc.sync.dma_start(qc_sb[:], qc_d.ap())
                p_qct = psA.tile([3, NQL], F32, tag="pq")
                nc.tensor.transpose(p_qct[:], qc_sb[:], ident[0:NQL, 0:NQL])
                qct_sb = tmpA.tile([3, NQL], F32, tag="qct_sb")
                nc.scalar.copy(qct_sb[:], p_qct[0:3, :])
                nc.sync.dma_start(s_qc.ap(), qct_sb[:])
                qcrep = tmpA.tile([128, NQL], F32, tag="qcrep")
                nc.gpsimd.dma_start(
                    qcrep[0:3 * len(CO_QNODES), :],
                    s_qc.ap().unsqueeze(0).broadcast_to(
                        [len(CO_QNODES), 3, NQL]))

                # kv tiles
                kvT_all = bigp.tile([128, (NT // 2) * 128], F32,
                                    tag="kvT_all")
                for pr in range(NT // 2):
                    p_t1 = psA.tile([128, 128], F32, tag="pt1")
                    nc.tensor.transpose(p_t1[:],
                                        kv_sb[:, pr * 128:(pr + 1) * 128],
                                        ident[:])
                    nc.vector.tensor_copy(
                        kvT_all[:, pr * 128:(pr + 1) * 128], p_t1[:])
                for t in range(NT):
                    pr, sx = t // 2, t % 2
                    kvT = kvT_all[64 * sx:64 * sx + 64,
                                  pr * 128:(pr + 1) * 128]
                    wkv_t = wkv[:] if sx == 0 else wkv_hi[64:128, :]
                    p_kv = psA.tile([128, 2 * C], F32, tag="pkv")
                    nc.tensor.matmul(p_kv[:], kvT, wkv_t, start=True,
                                     stop=True)
                    vbase = t * 65
                    nc.scalar.copy(vext[:, vbase:vbase + C],
                                   p_kv[:, C:2 * C])
                    r0 = t * 128
                    if t < NT - 2:
                        nc.gpsimd.memset(vext[:, vbase + C:vbase + 65], 1.0)
                    elif t == NT - 2:
                        nc.gpsimd.memset(vext[:, vbase + C:vbase + 65], 0.0)
                        nc.gpsimd.memset(
                            vext[0:NKV - r0, vbase + C:vbase + 65], 1.0)
                    else:
                        nc.gpsimd.memset(vext[:, vbase + C:vbase + 65], 0.0)
                    p_kt = psA.tile([128, 128], F32, tag="pkt")
                    if t < NCH:
                        wkv_k = (wkv[:, 0:C] if sx == 0
                                 else wkv_hi[64:128, 0:C])
                        nc.tensor.matmul(p_kt[0:C, :], wkv_k, kvT,
                                         start=True, stop=True)
                        nc.vector.tensor_copy(
                            kT2[0:64, t * 128:(t + 1) * 128], p_kt[0:C, :])
                    else:
                        wkvkz_t = (wkvkz[:] if sx == 0
                                   else wkvkz_hi[64:128, :])
                        nc.tensor.matmul(p_kt[:], wkvkz_t, kvT,
                                         start=True, stop=True)
                        c0 = (t - NCH) * 128
                        nc.vector.tensor_copy(kT2[64:128, c0:c0 + 128],
                                              p_kt[64:128, :])

                # ---------- q-side packs + mixing ----------
                qf = [tmpA.tile([128, NQL], F32, tag=f"qf{p}",
                                name=f"qf{p}") for p in range(NQP)]
                nc.gpsimd.memset(qf[0][0:64, :], 1.0)
                nc.vector.tensor_copy(qf[0][64:128, :], qT2[0:64, :])
                nc.vector.memset(qf[1][64:128, :], 0.0)
                nc.scalar.activation(qf[1][0:64, :], qT2[0:64, :],
                                     ACTF.Square)
                nc.scalar.activation(qf[2][:], qT2[:], ACTF.Exp,
                                     scale=exps[:])
                for i in range(7):
                    nc.scalar.activation(qf[3 + i][:], qT2[:], ACTF.Tanh,
                                         scale=tnsb[i][0][:],
                                         bias=tnsb[i][1][:])
                for i in range(4):
                    nc.vector.tensor_scalar(qf[10 + i][:], qT2[:],
                                            absc[i][:], None, ALU.subtract)
                    nc.vector.tensor_scalar(qf[10 + i][:].bitcast(U32),
                                            qf[10 + i][:].bitcast(U32),
                                            0x7FFFFFFF, None,
                                            ALU.bitwise_and)

                with tc.tile_pool(name="psM", bufs=1, space=PSUM) as psM:
                    for r in range(NKP):
                        pmix = psM.tile([128, NQL], F32, tag="pmix",
                                        name=f"pmix{r}", bufs=2)
                        for p in range(NQP):
                            base = (r * NQP + p) * 128
                            nc.tensor.matmul(
                                pmix[:], bigw[:, base:base + 128],
                                qf[p][:], start=(p == 0), stop=(p == NQP - 1))
                        n0, n1 = 2 * r, 2 * r + 1
                        nc.gpsimd.memset(fq[n0][0:64, 64:128], 0.0)
                        nc.gpsimd.memset(fq[n0][64:128, 0:64], 0.0)
                        nc.vector.tensor_copy(fq[n0][0:64, 0:64],
                                              pmix[0:64, :])
                        nc.gpsimd.tensor_copy(fq[n0][64:128, 64:128],
                                              fq[n0][0:64, 0:64])
                        if n1 < SK:
                            nc.gpsimd.memset(fq[n1][0:64, 64:128], 0.0)
                            nc.gpsimd.memset(fq[n1][64:128, 0:64], 0.0)
                            nc.vector.tensor_copy(fq[n1][0:64, 0:64],
                                                  pmix[64:128, :])
                            nc.gpsimd.tensor_copy(fq[n1][64:128, 64:128],
                                                  fq[n1][0:64, 0:64])

                    # coord q-side: rows [0:30] |qc-t|, [32:35] ones
                    qcf_bf = tmpA.tile([NCOQ, NQL], F32, tag="qcfb")
                    nq_abs = 3 * len(CO_QNODES)
                    nc.vector.memset(qcf_bf[:], 0.0)
                    nc.vector.tensor_scalar(qcf_bf[0:nq_abs, :],
                                            qcrep[0:nq_abs, :],
                                            coqt[0:nq_abs, :], None,
                                            ALU.subtract)
                    nc.vector.tensor_scalar(qcf_bf[0:nq_abs, :].bitcast(U32),
                                            qcf_bf[0:nq_abs, :].bitcast(U32),
                                            0x7FFFFFFF, None,
                                            ALU.bitwise_and)
                    nc.vector.memset(qcf_bf[32:35, :], 1.0)
                    pco = psM.tile([NCOR, NQL], F32, tag="pco")
                    nc.tensor.matmul(pco[:], cwb[:], qcf_bf[:],
                                     start=True, stop=True)
                    acof = tmpA.tile([NCOR, NQL], F32, tag="acof")
                    nc.vector.tensor_scalar(acof[:], pco[:], wrep[:], 0.0,
                                            ALU.mult)
                    nc.gpsimd.memset(aco[:], 0.0)
                    nc.vector.tensor_copy(aco[0:NCOR, 0:64], acof[:])
                    nc.vector.tensor_copy(aco[64:64 + NCOR, 64:128], acof[:])

            # ------------- k-side features -------------
            nc.scalar.activation(glist[1][:], kT2[:], ACTF.Square)
            nc.scalar.activation(glist[2][:], kT2[:], ACTF.Exp, scale=1.0)
            nc.scalar.activation(glist[3][:], kT2[:], ACTF.Exp, scale=-1.0)
            for i, (s, c) in enumerate(PSI_TANH):
                nc.scalar.activation(glist[4 + i][:], kT2[:], ACTF.Tanh,
                                     scale=float(s), bias=kpbt[:, i:i + 1])
            for i, c in enumerate(PSI_ABS):
                nc.vector.tensor_scalar(glist[10 + i][:], kT2[:], float(c),
                                        None, ALU.subtract)
                nc.vector.tensor_scalar(glist[10 + i][:].bitcast(U16),
                                        glist[10 + i][:].bitcast(U16),
                                        0x7FFF, None, ALU.bitwise_and)
            nc.vector.memset(gco[32:64, :], 0.0)
            nc.vector.memset(gco[96:128, :], 0.0)
            for rr in (slice(0, NCOR), slice(64, 64 + NCOR)):
                nc.vector.tensor_scalar(gco[rr, :], kcrep[rr, :],
                                        cokt[rr, :], None, ALU.subtract)
                nc.vector.tensor_scalar(gco[rr, :].bitcast(U16),
                                        gco[rr, :].bitcast(U16),
                                        0x7FFF, None, ALU.bitwise_and)

            # ------------- score + tail -------------
            with tc.tile_pool(name="psS", bufs=1, space=PSUM) as psS, \
                 tc.tile_pool(name="psT", bufs=2, space=PSUM) as psT, \
                 tc.tile_pool(name="psC", bufs=1, space=PSUM) as psC:
                import os
                skip_ch = os.environ.get("K2_SKIP_CH") == "1"
                skip_co = os.environ.get("K2_SKIP_CO") == "1"
                pscore = psS.tile([128, NH], F32, tag="pscore")
                nc.vector.memset(pscore[:], 0.0)
                regions = [(0, 512), (512, 1024), (1024, 1408)]
                p_att = psC.tile([NQL, 65], F32, tag="patt")
                for ri, (c0, c1) in enumerate(regions):
                    if not skip_ch:
                        for n in range(SK):
                            nc.tensor.matmul(
                                pscore[:, c0:c1], fq[n][:, :],
                                glist[n][:, c0:c1],
                                start=False, stop=False,
                                skip_group_check=True)
                    if not skip_co:
                        nc.tensor.matmul(
                            pscore[:, c0:c1], aco[:, :], gco[:, c0:c1],
                            start=False, stop=True, skip_group_check=True)
                for ri, (c0, c1) in enumerate(regions):
                    # tail for this region: exp -> transpose -> attn@[v|1]
                    nc.scalar.activation(e2sb[:, c0:c1], pscore[:, c0:c1],
                                         ACTF.Exp, scale=SCALE)
                    for jc in range(c0 // 128, (c1 + 127) // 128):
                        for h in (0, 1):
                            t = jc + NCH * h
                            src_ = e2sb[64 * h:64 * h + 64,
                                        jc * 128:(jc + 1) * 128]
                            p_tr = psT.tile([128, NQL], BF16, tag="ptr")
                            nc.tensor.transpose(
                                p_tr[:], src_,
                                identb[64 * h:64 * h + 64,
                                       64 * h:64 * h + 64])
                            eT = wp.tile([128, NQL], BF16, tag="eT")
                            nc.vector.tensor_copy(eT[:], p_tr[:])
                            nc.tensor.matmul(p_att[:], eT[:],
                                             vext[:, t * 65:(t + 1) * 65],
                                             start=(t == 0), stop=(t == NT - 1),
                                             skip_group_check=True)

                rs = wp.tile([NQL, 1], F32, tag="rs")
                nc.vector.reciprocal(rs[:], p_att[:, C:C + 1])
                o_sb = wp.tile([NQL, C], F32, tag="o_sb")
                nc.vector.tensor_scalar(o_sb[:], p_att[:, 0:C], rs[:], 0.0,
                                        ALU.mult)
                p_ot = psC.tile([C, NQL], F32, tag="pp")
                nc.tensor.transpose(p_ot[:], o_sb[:], ident[0:NQL, 0:NQL])
                ot_sb = wp.tile([C, NQL], F32, tag="ot_sb")
                nc.vector.tensor_copy(ot_sb[:], p_ot[:])
                p_pj = psC.tile([C, NQL], F32, tag="pp")
                nc.tensor.matmul(p_pj[:], wproj[:], ot_sb[:], start=True,
                                 stop=True)
                pj_sb = wp.tile([C, NQL], F32, tag="pj_sb")
                nc.vector.tensor_scalar(pj_sb[:], p_pj[:], bproj[:], 0.0,
                                        ALU.add)
                p_o2 = psC.tile([NQL, C], F32, tag="pp")
                nc.tensor.transpose(p_o2[:], pj_sb[:], ident[0:C, 0:C])
                po_sb = wp.tile([NQL, C], F32, tag="po_sb")
                nc.vector.tensor_copy(po_sb[:], p_o2[:])
                nc.sync.dma_start(out_d.ap(), po_sb[:])

    nc.compile()
    return nc


def _get_compiled():
    global _COMPILED
    if _COMPILED is None:
        _COMPILED = _build_program()
    return _COMPILED


def _make_in_maps(inputs):
    q = np.asarray(inputs["q"], np.float32).reshape(NQ, C)
    qc = np.asarray(inputs["q_coord"], np.float32).reshape(NQ, 3)
    kv = np.asarray(inputs["kv"], np.float32).reshape(NKV, ICO)
    kvc = np.asarray(inputs["kv_coord"], np.float32).reshape(NKV, 3)
    shared = {
        "kv": np.ascontiguousarray(kv),
        "kv_coord": np.ascontiguousarray(kvc),
        "Wq": np.ascontiguousarray(np.asarray(inputs["Wq"], np.float32)),
        "Wkv": np.ascontiguousarray(np.asarray(inputs["Wkv"], np.float32)),
        "Wdelta": np.ascontiguousarray(
            np.asarray(inputs["Wdelta"], np.float32)),
        "Wproj": np.ascontiguousarray(np.asarray(inputs["Wproj"], np.float32)),
        "bproj": np.ascontiguousarray(
            np.asarray(inputs["bproj"], np.float32).reshape(C, 1)),
    }
    in_maps = []
    for core in range(NCORES):
        sl = slice(core * NQL, (core + 1) * NQL)
        m = dict(shared)
        m["q"] = np.ascontiguousarray(q[sl])
        m["q_coord"] = np.ascontiguousarray(qc[sl])
        in_maps.append(m)
    return in_maps


def run_on_hw(inputs, trace=False, **kw):
    from concourse.bass_utils import run_bass_kernel_spmd

    nc = _get_compiled()
    in_maps = _make_in_maps(inputs)
    res = run_bass_kernel_spmd(nc, in_maps, list(range(NCORES)), trace=trace,
                               **kw)
    out = np.concatenate([r["out"] for r in res.results], axis=0)
    return out.reshape(B, NQ, C).astype(np.float32), res


def kernel(**inputs) -> np.ndarray:
    out, _ = run_on_hw(inputs, trace=False)
    return out
